# revision 3
# baseline (speedup 1.0000x reference)
"""2-layer GCN (segment-mean -> Linear -> ReLU -> segment-mean -> Linear) on
8 Trainium2 NeuronCores.

Strategy:
- Shard dst nodes (and their incident edges) across the 8 cores.
- Edges sorted by (dst tile, src chunk); src features fetched with
  dma_gather (4 parallel SWDGE queues) as bf16 rows.
- Segment-sum per 128-dst tile via one-hot matmuls accumulated in PSUM;
  one-hot matrices built in a single DVE is_equal op per tile.
- Linear layers fused per tile (W1 + ReLU + W2); the [N, 64] intermediate
  z = relu(...) @ W2 is exchanged between cores with an AllGather, so the
  second segment-mean gathers 256B rows locally.
- Uses the algebraic identity segmean(h) @ W = segmean(h @ W).
"""
import os
import numpy as np
import ml_dtypes

import concourse.bass as bass
import concourse.mybir as mybir
from concourse import bacc
from concourse.tile import TileContext
from concourse.masks import make_identity
from concourse.bass_utils import run_bass_kernel_spmd

P = 128
NCORES = 8
SRC_CHUNK = 25600           # int16-addressable table chunk
OP_CAP = 640                # max idxs per dma_gather op
BF16 = ml_dtypes.bfloat16

LAST_EXEC_NS = None


def _prep(src, dst, n_nodes):
    """Host-side graph preprocessing. Returns the uniform (SPMD) structure and
    per-core data arrays."""
    shard = n_nodes // NCORES                      # 12500
    n_tiles = (shard + P - 1) // P                 # 98
    n_buckets = (n_nodes + SRC_CHUNK - 1) // SRC_CHUNK  # 4

    src = np.asarray(src, np.int64)
    dst = np.asarray(dst, np.int64)

    per_core = []
    counts = np.zeros((NCORES, n_tiles, n_buckets), np.int64)
    for c in range(NCORES):
        m = (dst >= c * shard) & (dst < (c + 1) * shard)
        s = src[m]
        d = dst[m] - c * shard
        tile = d // P
        bucket = s // SRC_CHUNK
        key = tile * n_buckets + bucket
        order = np.argsort(key, kind="stable")
        s, d, tile, bucket, key = s[order], d[order], tile[order], bucket[order], key[order]
        cnt_tb = np.bincount(key, minlength=n_tiles * n_buckets).reshape(n_tiles, n_buckets)
        counts[c] = cnt_tb
        cnt_node = np.bincount(d, minlength=shard).astype(np.float64)
        per_core.append(dict(s=s, d=d, key=key, cnt_tb=cnt_tb, cnt_node=cnt_node))

    # uniform per-(tile,bucket) padded length across cores
    L_tb = ((counts.max(axis=0) + P - 1) // P) * P        # [n_tiles, n_buckets]
    empty = L_tb.sum(axis=1) == 0
    L_tb[empty, 0] = P                                    # all-dummy chunk for empty tiles
    nch_tb = L_tb // P
    nch_t = nch_tb.sum(axis=1)                            # chunks per tile
    total_chunks = int(nch_t.sum())
    total_slots = total_chunks * P
    chunk_off_tb = np.concatenate([[0], np.cumsum(nch_tb.reshape(-1))])[:-1].reshape(n_tiles, n_buckets)

    data = []
    for c in range(NCORES):
        pc = per_core[c]
        idx16 = np.zeros(total_slots, np.int16)           # pad -> row 0 of chunk
        dloc = np.full(total_slots, 999.0, np.float32)    # pad -> matches no column
        # place each (t,b) segment at its slot offset
        cnt_flat = pc["cnt_tb"].reshape(-1)
        seg_start_edges = np.concatenate([[0], np.cumsum(cnt_flat)])[:-1]
        slot_base = (chunk_off_tb.reshape(-1) * P)
        # within-segment position for each edge
        pos_in_seg = np.arange(len(pc["s"])) - seg_start_edges[pc["key"]]
        slots = slot_base[pc["key"]] + pos_in_seg
        idx16[slots] = (pc["s"] - (pc["s"] // SRC_CHUNK) * SRC_CHUNK).astype(np.int16)
        dloc[slots] = (pc["d"] % P).astype(np.float32)

        # wrapped idx layout [128, total_slots//16]
        w = idx16.reshape(-1, 16).T                       # [16, S/16]
        idx_wrapped = np.tile(w, (8, 1))                  # [128, S/16]
        # dstloc layout [128, total_chunks]
        dlsb = dloc.reshape(total_chunks, P).T.astype(BF16)

        cnt_node = pc["cnt_node"]
        recip = (1.0 / np.maximum(cnt_node, 1.0)).astype(np.float32)
        cntp = np.maximum(cnt_node, 1.0).astype(np.float32)
        recip_pad = np.ones(n_tiles * P, np.float32)
        recip_pad[:shard] = recip
        cntp_pad = np.ones(n_tiles * P, np.float32)
        cntp_pad[:shard] = cntp
        rb = np.broadcast_to(recip_pad.reshape(n_tiles, 1, P), (n_tiles, P, P)).astype(BF16)
        rc_col = recip_pad.reshape(n_tiles, P, 1).astype(np.float32)
        cr_row = cntp_pad.reshape(n_tiles, 1, P).astype(BF16)
        data.append(dict(idx=idx_wrapped, dlsb=dlsb, rb=np.ascontiguousarray(rb),
                         rc=rc_col, cr=cr_row))

    struct = dict(n_tiles=n_tiles, n_buckets=n_buckets, L_tb=L_tb, nch_t=nch_t,
                  chunk_off_tb=chunk_off_tb, total_chunks=total_chunks,
                  total_slots=total_slots, shard=shard)
    return struct, data


def _build(st, n_nodes):
    n_tiles, n_buckets = st["n_tiles"], st["n_buckets"]
    L_tb, nch_t = st["L_tb"], st["nch_t"]
    chunk_off_tb = st["chunk_off_tb"]
    TOTCH, SLOT16 = st["total_chunks"], st["total_slots"] // 16
    shard = st["shard"]
    max_nch = int(nch_t.max())
    f32, bf16, i16 = mybir.dt.float32, mybir.dt.bfloat16, mybir.dt.int16

    nc = bacc.Bacc("TRN2", target_bir_lowering=False, debug=False,
                   num_devices=NCORES, num_swdge_queues=4,
                   dynamic_dma_scratch_size=65536)
    X_d = nc.dram_tensor("X", [n_nodes, P], bf16, kind="ExternalInput")
    W1_d = nc.dram_tensor("W1b", [P, P], bf16, kind="ExternalInput")
    W2_d = nc.dram_tensor("W2b", [P, 64], bf16, kind="ExternalInput")
    b1_d = nc.dram_tensor("b1c", [P, 1], f32, kind="ExternalInput")
    b2_d = nc.dram_tensor("b2r", [1, 64], bf16, kind="ExternalInput")
    iota_d = nc.dram_tensor("iota", [P, P], bf16, kind="ExternalInput")
    idx_d = nc.dram_tensor("idx", [P, SLOT16], i16, kind="ExternalInput")
    dl_d = nc.dram_tensor("dl", [P, TOTCH], bf16, kind="ExternalInput")
    rb_d = nc.dram_tensor("rb", [n_tiles, P, P], bf16, kind="ExternalInput")
    rc_d = nc.dram_tensor("rc", [n_tiles, P, 1], f32, kind="ExternalInput")
    cr_d = nc.dram_tensor("cr", [n_tiles, 1, P], bf16, kind="ExternalInput")
    out_d = nc.dram_tensor("out", [shard, 64], f32, kind="ExternalOutput")

    z_local = nc.dram_tensor("z_local", [shard, P], bf16)
    z_full = nc.dram_tensor("z_full", [NCORES * shard, P], bf16, addr_space="Shared")

    qn = [0]

    with TileContext(nc) as tc:
        with tc.tile_pool(name="const", bufs=1) as cpool, \
             tc.tile_pool(name="g", bufs=4) as gpool, \
             tc.tile_pool(name="oh", bufs=4) as ohpool, \
             tc.tile_pool(name="wk", bufs=3) as wpool, \
             tc.tile_pool(name="sm", bufs=3) as smpool, \
             tc.tile_pool(name="ps1", bufs=2, space="PSUM") as ps1, \
             tc.tile_pool(name="ps2", bufs=2, space="PSUM") as ps2, \
             tc.tile_pool(name="ps3", bufs=2, space="PSUM") as ps3, \
             tc.tile_pool(name="ps4", bufs=2, space="PSUM") as ps4:

            W1sb = cpool.tile([P, P], bf16)
            nc.sync.dma_start(out=W1sb[:], in_=W1_d[:])
            W2sb = cpool.tile([P, 64], bf16)
            nc.sync.dma_start(out=W2sb[:], in_=W2_d[:])
            b1sb = cpool.tile([P, 1], f32)
            nc.sync.dma_start(out=b1sb[:], in_=b1_d[:])
            b2sb = cpool.tile([1, 64], bf16)
            nc.sync.dma_start(out=b2sb[:], in_=b2_d[:])
            iotasb = cpool.tile([P, P], bf16)
            nc.sync.dma_start(out=iotasb[:], in_=iota_d[:])
            idxsb = cpool.tile([P, SLOT16], i16)
            nc.sync.dma_start(out=idxsb[:], in_=idx_d[:])
            dlsb = cpool.tile([P, TOTCH], bf16)
            nc.sync.dma_start(out=dlsb[:], in_=dl_d[:])
            ident = cpool.tile([P, P], bf16)
            make_identity(nc, ident[:])

            for layer in (0, 1):
                table = X_d if layer == 0 else z_full
                for t in range(n_tiles):
                    nch = int(nch_t[t])
                    G = gpool.tile([P, max_nch * P], bf16, tag="G")
                    for b in range(n_buckets):
                        L = int(L_tb[t, b])
                        if L == 0:
                            continue
                        co = int(chunk_off_tb[t, b] - chunk_off_tb[t, 0])
                        gco = int(chunk_off_tb[t, b])
                        nc.gpsimd.dma_gather(
                            G[:, co * P:(co + L // P) * P].rearrange("p (c d) -> p c d", d=P),
                            table[b * SRC_CHUNK:min((b + 1) * SRC_CHUNK, n_nodes), :],
                            idxsb[:, gco * 8:gco * 8 + L // 16],
                            L, L, P,
                            queue_num=qn[0] % 4,
                        )
                        qn[0] += 1
                    oh = ohpool.tile([P, max_nch * P], bf16, tag="oh")
                    dcol0 = int(chunk_off_tb[t, 0])
                    in0 = iotasb[:].rearrange("p (o d) -> p o d", o=1).broadcast_to([P, nch, P])
                    in1 = dlsb[:, dcol0:dcol0 + nch].rearrange("p (c o) -> p c o", o=1).broadcast_to([P, nch, P])
                    nc.vector.tensor_tensor(
                        out=oh[:, :nch * P].rearrange("p (c d) -> p c d", d=P),
                        in0=in0, in1=in1, op=mybir.AluOpType.is_equal)
                    psum1 = ps1.tile([P, P], f32, space="PSUM", tag="p1")
                    for cci in range(nch):
                        nc.tensor.matmul(
                            out=psum1[:], lhsT=G[:, cci * P:(cci + 1) * P],
                            rhs=oh[:, cci * P:(cci + 1) * P],
                            start=(cci == 0), stop=(cci == nch - 1))
                    rows = min(P, shard - t * P)
                    if layer == 0:
                        rbt = smpool.tile([P, P], bf16, tag="rbt")
                        nc.sync.dma_start(out=rbt[:], in_=rb_d[t])
                        m1 = wpool.tile([P, P], bf16, tag="m1")
                        nc.vector.tensor_tensor(out=m1[:], in0=psum1[:], in1=rbt[:],
                                                op=mybir.AluOpType.mult)
                        psum2 = ps2.tile([P, P], f32, space="PSUM", tag="p2")
                        nc.tensor.matmul(out=psum2[:], lhsT=W1sb[:], rhs=m1[:],
                                         start=True, stop=True)
                        h1T = wpool.tile([P, P], bf16, tag="h1T")
                        nc.scalar.activation(out=h1T[:], in_=psum2[:],
                                             func=mybir.ActivationFunctionType.Relu,
                                             bias=b1sb[:, :1], scale=1.0)
                        psum3 = ps3.tile([64, P], f32, space="PSUM", tag="p3")
                        nc.tensor.matmul(out=psum3[:], lhsT=W2sb[:], rhs=h1T[:],
                                         start=True, stop=True)
                        zT = wpool.tile([64, P], bf16, tag="zT")
                        nc.scalar.activation(out=zT[:], in_=psum3[:],
                                             func=mybir.ActivationFunctionType.Copy,
                                             scale=1.0)
                        psum4 = ps4.tile([P, 64], f32, space="PSUM", tag="p4")
                        nc.tensor.matmul(out=psum4[:], lhsT=zT[:], rhs=ident[:64, :64],
                                         start=True, stop=True)
                        zt = wpool.tile([P, 64], bf16, tag="zt")
                        nc.scalar.activation(out=zt[:], in_=psum4[:],
                                             func=mybir.ActivationFunctionType.Copy,
                                             scale=1.0)
                        nc.sync.dma_start(out=z_local[t * P:t * P + rows, :64],
                                          in_=zt[:rows, :])
                    else:
                        s5 = wpool.tile([64, P], bf16, tag="zT")
                        nc.scalar.activation(out=s5[:], in_=psum1[:64, :],
                                             func=mybir.ActivationFunctionType.Copy,
                                             scale=1.0)
                        psum4b = ps4.tile([P, 64], f32, space="PSUM", tag="p4")
                        nc.tensor.matmul(out=psum4b[:], lhsT=s5[:], rhs=ident[:64, :64],
                                         start=True, stop=False)
                        crt = smpool.tile([1, P], bf16, tag="crt")
                        nc.sync.dma_start(out=crt[:], in_=cr_d[t])
                        nc.tensor.matmul(out=psum4b[:], lhsT=crt[:], rhs=b2sb[:],
                                         start=False, stop=True)
                        rct = smpool.tile([P, 1], f32, tag="rct")
                        nc.sync.dma_start(out=rct[:], in_=rc_d[t])
                        outt = wpool.tile([P, 64], f32, tag="outt")
                        nc.scalar.activation(out=outt[:], in_=psum4b[:],
                                             func=mybir.ActivationFunctionType.Copy,
                                             scale=rct[:, :1])
                        nc.sync.dma_start(out=out_d[t * P:t * P + rows, :],
                                          in_=outt[:rows, :])
                if layer == 0:
                    nc.gpsimd.collective_compute(
                        "AllGather", mybir.AluOpType.bypass,
                        replica_groups=[list(range(NCORES))],
                        ins=[z_local[:]], outs=[z_full[:]])
    nc.compile()
    return nc


def _gcn(features, W1, b1, W2, b2, src, dst):
    global LAST_EXEC_NS
    n_nodes = features.shape[0]
    st, data = _prep(src, dst, n_nodes)

    X16 = np.ascontiguousarray(np.asarray(features, np.float32)).astype(BF16)
    iota_host = np.tile(np.arange(P, dtype=np.float32)[None, :], (P, 1)).astype(BF16)
    common = dict(
        X=X16,
        W1b=np.asarray(W1, np.float32).astype(BF16),
        W2b=np.asarray(W2, np.float32).astype(BF16),
        b1c=np.asarray(b1, np.float32).reshape(P, 1),
        b2r=np.asarray(b2, np.float32).reshape(1, 64).astype(BF16),
        iota=iota_host,
    )
    in_maps = []
    for c in range(NCORES):
        d = data[c]
        in_maps.append(dict(common, idx=d["idx"], dl=d["dlsb"], rb=d["rb"],
                            rc=d["rc"], cr=d["cr"]))

    nc = _build(st, n_nodes)
    trace = bool(os.environ.get("GCN_TRACE"))
    try:
        res = run_bass_kernel_spmd(nc, in_maps, list(range(NCORES)), trace=trace)
    except Exception:
        if not trace:
            raise
        res = run_bass_kernel_spmd(nc, in_maps, list(range(NCORES)))
    LAST_EXEC_NS = res.exec_time_ns
    out = np.concatenate([res.results[c]["out"] for c in range(NCORES)], axis=0)
    return np.ascontiguousarray(out, dtype=np.float32)


def kernel(features, W1, b1, W2, b2, src, dst):
    return _gcn(features, W1, b1, W2, b2, src, dst)


# revision 4
# speedup vs baseline: 1.0282x; 1.0282x over previous
"""2-layer GCN (segment-mean -> Linear -> ReLU -> segment-mean -> Linear) on
8 Trainium2 NeuronCores.

Strategy:
- Shard dst nodes (and their incident edges) across the 8 cores.
- Edges sorted by (dst tile, src chunk); src features fetched with
  dma_gather (4 parallel SWDGE queues) as bf16 rows.
- Segment-sum per 128-dst tile via one-hot matmuls accumulated in PSUM;
  one-hot matrices built in a single DVE is_equal op per tile.
- Linear layers fused per tile (W1 + ReLU + W2); the [N, 64] intermediate
  z = relu(...) @ W2 is exchanged between cores with an AllGather, so the
  second segment-mean gathers 256B rows locally.
- Uses the algebraic identity segmean(h) @ W = segmean(h @ W).
"""
import os
import numpy as np
import ml_dtypes

import concourse.bass as bass
import concourse.mybir as mybir
from concourse import bacc
from concourse.tile import TileContext
from concourse.masks import make_identity
from concourse.bass_utils import run_bass_kernel_spmd

P = 128
NCORES = 8
SRC_CHUNK = 25600           # int16-addressable table chunk
OP_CAP = 640                # max idxs per dma_gather op
BF16 = ml_dtypes.bfloat16

LAST_EXEC_NS = None


def _prep(src, dst, n_nodes):
    """Host-side graph preprocessing. Returns the uniform (SPMD) structure and
    per-core data arrays."""
    shard = n_nodes // NCORES                      # 12500
    n_tiles = (shard + P - 1) // P                 # 98
    n_buckets = (n_nodes + SRC_CHUNK - 1) // SRC_CHUNK  # 4

    src = np.asarray(src, np.int64)
    dst = np.asarray(dst, np.int64)

    per_core = []
    counts = np.zeros((NCORES, n_tiles, n_buckets), np.int64)
    for c in range(NCORES):
        m = (dst >= c * shard) & (dst < (c + 1) * shard)
        s = src[m]
        d = dst[m] - c * shard
        tile = d // P
        bucket = s // SRC_CHUNK
        key = tile * n_buckets + bucket
        order = np.argsort(key, kind="stable")
        s, d, tile, bucket, key = s[order], d[order], tile[order], bucket[order], key[order]
        cnt_tb = np.bincount(key, minlength=n_tiles * n_buckets).reshape(n_tiles, n_buckets)
        counts[c] = cnt_tb
        cnt_node = np.bincount(d, minlength=shard).astype(np.float64)
        per_core.append(dict(s=s, d=d, key=key, cnt_tb=cnt_tb, cnt_node=cnt_node))

    # uniform per-(tile,bucket) padded length across cores
    Lt_true = counts.max(axis=0)                          # [n_tiles, n_buckets]
    L_tb = ((Lt_true + P - 1) // P) * P
    empty = L_tb.sum(axis=1) == 0
    L_tb[empty, 0] = P                                    # all-dummy chunk for empty tiles
    Lt_true = np.maximum(Lt_true, (L_tb > 0).astype(np.int64))
    nch_tb = L_tb // P
    nch_t = nch_tb.sum(axis=1)                            # chunks per tile
    total_chunks = int(nch_t.sum())
    total_slots = total_chunks * P
    chunk_off_tb = np.concatenate([[0], np.cumsum(nch_tb.reshape(-1))])[:-1].reshape(n_tiles, n_buckets)

    data = []
    for c in range(NCORES):
        pc = per_core[c]
        idx16 = np.zeros(total_slots, np.int16)           # pad -> row 0 of chunk
        dloc = np.full(total_slots, 999.0, np.float32)    # pad -> matches no column
        # place each (t,b) segment at its slot offset
        cnt_flat = pc["cnt_tb"].reshape(-1)
        seg_start_edges = np.concatenate([[0], np.cumsum(cnt_flat)])[:-1]
        slot_base = (chunk_off_tb.reshape(-1) * P)
        # within-segment position for each edge
        pos_in_seg = np.arange(len(pc["s"])) - seg_start_edges[pc["key"]]
        slots = slot_base[pc["key"]] + pos_in_seg
        idx16[slots] = (pc["s"] - (pc["s"] // SRC_CHUNK) * SRC_CHUNK).astype(np.int16)
        dloc[slots] = (pc["d"] % P).astype(np.float32)

        # wrapped idx layout [128, total_slots//16]
        w = idx16.reshape(-1, 16).T                       # [16, S/16]
        idx_wrapped = np.tile(w, (8, 1))                  # [128, S/16]
        # dstloc layout [128, total_chunks]
        dlsb = dloc.reshape(total_chunks, P).T.astype(BF16)

        cnt_node = pc["cnt_node"]
        recip = (1.0 / np.maximum(cnt_node, 1.0)).astype(np.float32)
        cntp = np.maximum(cnt_node, 1.0).astype(np.float32)
        recip_pad = np.ones(n_tiles * P, np.float32)
        recip_pad[:shard] = recip
        cntp_pad = np.ones(n_tiles * P, np.float32)
        cntp_pad[:shard] = cntp
        rb = np.broadcast_to(recip_pad.reshape(n_tiles, 1, P), (n_tiles, P, P)).astype(BF16)
        rc_col = recip_pad.reshape(n_tiles, P, 1).astype(np.float32)
        cr_row = cntp_pad.reshape(n_tiles, 1, P).astype(BF16)
        data.append(dict(idx=idx_wrapped, dlsb=dlsb, rb=np.ascontiguousarray(rb),
                         rc=rc_col, cr=cr_row))

    struct = dict(n_tiles=n_tiles, n_buckets=n_buckets, L_tb=L_tb, nch_t=nch_t, Lt_true=Lt_true,
                  chunk_off_tb=chunk_off_tb, total_chunks=total_chunks,
                  total_slots=total_slots, shard=shard)
    return struct, data


def _build(st, n_nodes):
    n_tiles, n_buckets = st["n_tiles"], st["n_buckets"]
    L_tb, nch_t, Lt_true = st["L_tb"], st["nch_t"], st["Lt_true"]
    chunk_off_tb = st["chunk_off_tb"]
    TOTCH, SLOT16 = st["total_chunks"], st["total_slots"] // 16
    shard = st["shard"]
    max_nch = int(nch_t.max())
    f32, bf16, i16 = mybir.dt.float32, mybir.dt.bfloat16, mybir.dt.int16

    nc = bacc.Bacc("TRN2", target_bir_lowering=False, debug=False,
                   num_devices=NCORES, num_swdge_queues=4,
                   dynamic_dma_scratch_size=65536)
    X_d = nc.dram_tensor("X", [n_nodes, P], bf16, kind="ExternalInput")
    W1_d = nc.dram_tensor("W1b", [P, P], bf16, kind="ExternalInput")
    W2_d = nc.dram_tensor("W2b", [P, 64], bf16, kind="ExternalInput")
    b1_d = nc.dram_tensor("b1c", [P, 1], f32, kind="ExternalInput")
    b2_d = nc.dram_tensor("b2r", [1, 64], bf16, kind="ExternalInput")
    iota_d = nc.dram_tensor("iota", [P, P], bf16, kind="ExternalInput")
    idx_d = nc.dram_tensor("idx", [P, SLOT16], i16, kind="ExternalInput")
    dl_d = nc.dram_tensor("dl", [P, TOTCH], bf16, kind="ExternalInput")
    rb_d = nc.dram_tensor("rb", [n_tiles, P, P], bf16, kind="ExternalInput")
    rc_d = nc.dram_tensor("rc", [n_tiles, P, 1], f32, kind="ExternalInput")
    cr_d = nc.dram_tensor("cr", [n_tiles, 1, P], bf16, kind="ExternalInput")
    out_d = nc.dram_tensor("out", [shard, 64], f32, kind="ExternalOutput")

    z_local = nc.dram_tensor("z_local", [shard, P], bf16)
    z_full = nc.dram_tensor("z_full", [NCORES * shard, P], bf16, addr_space="Shared")

    qn = [0]

    with TileContext(nc) as tc:
        with tc.tile_pool(name="const", bufs=1) as cpool, \
             tc.tile_pool(name="g", bufs=6) as gpool, \
             tc.tile_pool(name="oh", bufs=6) as ohpool, \
             tc.tile_pool(name="wk", bufs=3) as wpool, \
             tc.tile_pool(name="sm", bufs=3) as smpool, \
             tc.tile_pool(name="ps1", bufs=2, space="PSUM") as ps1, \
             tc.tile_pool(name="ps2", bufs=2, space="PSUM") as ps2, \
             tc.tile_pool(name="ps3", bufs=2, space="PSUM") as ps3, \
             tc.tile_pool(name="ps4", bufs=2, space="PSUM") as ps4:

            W1sb = cpool.tile([P, P], bf16)
            nc.sync.dma_start(out=W1sb[:], in_=W1_d[:])
            W2sb = cpool.tile([P, 64], bf16)
            nc.sync.dma_start(out=W2sb[:], in_=W2_d[:])
            b1sb = cpool.tile([P, 1], f32)
            nc.sync.dma_start(out=b1sb[:], in_=b1_d[:])
            b2sb = cpool.tile([1, 64], bf16)
            nc.sync.dma_start(out=b2sb[:], in_=b2_d[:])
            iotasb = cpool.tile([P, P], bf16)
            nc.sync.dma_start(out=iotasb[:], in_=iota_d[:])
            idxsb = cpool.tile([P, SLOT16], i16)
            nc.sync.dma_start(out=idxsb[:], in_=idx_d[:])
            dlsb = cpool.tile([P, TOTCH], bf16)
            nc.sync.dma_start(out=dlsb[:], in_=dl_d[:])
            ident = cpool.tile([P, P], bf16)
            make_identity(nc, ident[:])

            for layer in (0, 1):
                table = X_d if layer == 0 else z_full
                for t in range(n_tiles):
                    nch = int(nch_t[t])
                    G = gpool.tile([P, max_nch * P], bf16, tag="G")
                    for b in range(n_buckets):
                        L = int(L_tb[t, b])
                        if L == 0:
                            continue
                        co = int(chunk_off_tb[t, b] - chunk_off_tb[t, 0])
                        gco = int(chunk_off_tb[t, b])
                        # first few tiles emit full padded counts so G pool
                        # buffers never expose uninitialized SBUF to the MMs
                        ntrue = L if (layer == 0 and t < 6) else int(Lt_true[t, b])
                        ncols = (ntrue + 15) // 16
                        nc.gpsimd.dma_gather(
                            G[:, co * P:(co + L // P) * P].rearrange("p (c d) -> p c d", d=P),
                            table[b * SRC_CHUNK:min((b + 1) * SRC_CHUNK, n_nodes), :],
                            idxsb[:, gco * 8:gco * 8 + ncols],
                            ntrue, ntrue, P,
                            queue_num=qn[0] % 4,
                        )
                        qn[0] += 1
                    oh = ohpool.tile([P, max_nch * P], bf16, tag="oh")
                    dcol0 = int(chunk_off_tb[t, 0])
                    in0 = iotasb[:].rearrange("p (o d) -> p o d", o=1).broadcast_to([P, nch, P])
                    in1 = dlsb[:, dcol0:dcol0 + nch].rearrange("p (c o) -> p c o", o=1).broadcast_to([P, nch, P])
                    nc.vector.tensor_tensor(
                        out=oh[:, :nch * P].rearrange("p (c d) -> p c d", d=P),
                        in0=in0, in1=in1, op=mybir.AluOpType.is_equal)
                    psum1 = ps1.tile([P, P], f32, space="PSUM", tag="p1")
                    for cci in range(nch):
                        nc.tensor.matmul(
                            out=psum1[:], lhsT=G[:, cci * P:(cci + 1) * P],
                            rhs=oh[:, cci * P:(cci + 1) * P],
                            start=(cci == 0), stop=(cci == nch - 1))
                    rows = min(P, shard - t * P)
                    if layer == 0:
                        rbt = smpool.tile([P, P], bf16, tag="rbt")
                        nc.sync.dma_start(out=rbt[:], in_=rb_d[t])
                        m1 = wpool.tile([P, P], bf16, tag="m1")
                        nc.vector.tensor_tensor(out=m1[:], in0=psum1[:], in1=rbt[:],
                                                op=mybir.AluOpType.mult)
                        psum2 = ps2.tile([P, P], f32, space="PSUM", tag="p2")
                        nc.tensor.matmul(out=psum2[:], lhsT=W1sb[:], rhs=m1[:],
                                         start=True, stop=True)
                        h1T = wpool.tile([P, P], bf16, tag="h1T")
                        nc.scalar.activation(out=h1T[:], in_=psum2[:],
                                             func=mybir.ActivationFunctionType.Relu,
                                             bias=b1sb[:, :1], scale=1.0)
                        psum3 = ps3.tile([64, P], f32, space="PSUM", tag="p3")
                        nc.tensor.matmul(out=psum3[:], lhsT=W2sb[:], rhs=h1T[:],
                                         start=True, stop=True)
                        zT = wpool.tile([64, P], bf16, tag="zT")
                        nc.scalar.activation(out=zT[:], in_=psum3[:],
                                             func=mybir.ActivationFunctionType.Copy,
                                             scale=1.0)
                        psum4 = ps4.tile([P, 64], f32, space="PSUM", tag="p4")
                        nc.tensor.matmul(out=psum4[:], lhsT=zT[:], rhs=ident[:64, :64],
                                         start=True, stop=True)
                        zt = wpool.tile([P, 64], bf16, tag="zt")
                        nc.scalar.activation(out=zt[:], in_=psum4[:],
                                             func=mybir.ActivationFunctionType.Copy,
                                             scale=1.0)
                        nc.sync.dma_start(out=z_local[t * P:t * P + rows, :64],
                                          in_=zt[:rows, :])
                    else:
                        s5 = wpool.tile([64, P], bf16, tag="zT")
                        nc.scalar.activation(out=s5[:], in_=psum1[:64, :],
                                             func=mybir.ActivationFunctionType.Copy,
                                             scale=1.0)
                        psum4b = ps4.tile([P, 64], f32, space="PSUM", tag="p4")
                        nc.tensor.matmul(out=psum4b[:], lhsT=s5[:], rhs=ident[:64, :64],
                                         start=True, stop=False)
                        crt = smpool.tile([1, P], bf16, tag="crt")
                        nc.sync.dma_start(out=crt[:], in_=cr_d[t])
                        nc.tensor.matmul(out=psum4b[:], lhsT=crt[:], rhs=b2sb[:],
                                         start=False, stop=True)
                        rct = smpool.tile([P, 1], f32, tag="rct")
                        nc.sync.dma_start(out=rct[:], in_=rc_d[t])
                        outt = wpool.tile([P, 64], f32, tag="outt")
                        nc.scalar.activation(out=outt[:], in_=psum4b[:],
                                             func=mybir.ActivationFunctionType.Copy,
                                             scale=rct[:, :1])
                        nc.sync.dma_start(out=out_d[t * P:t * P + rows, :],
                                          in_=outt[:rows, :])
                if layer == 0:
                    nc.gpsimd.collective_compute(
                        "AllGather", mybir.AluOpType.bypass,
                        replica_groups=[list(range(NCORES))],
                        ins=[z_local[:]], outs=[z_full[:]])
    nc.compile()
    return nc


def _gcn(features, W1, b1, W2, b2, src, dst):
    global LAST_EXEC_NS
    n_nodes = features.shape[0]
    st, data = _prep(src, dst, n_nodes)

    X16 = np.ascontiguousarray(np.asarray(features, np.float32)).astype(BF16)
    iota_host = np.tile(np.arange(P, dtype=np.float32)[None, :], (P, 1)).astype(BF16)
    common = dict(
        X=X16,
        W1b=np.asarray(W1, np.float32).astype(BF16),
        W2b=np.asarray(W2, np.float32).astype(BF16),
        b1c=np.asarray(b1, np.float32).reshape(P, 1),
        b2r=np.asarray(b2, np.float32).reshape(1, 64).astype(BF16),
        iota=iota_host,
    )
    in_maps = []
    for c in range(NCORES):
        d = data[c]
        in_maps.append(dict(common, idx=d["idx"], dl=d["dlsb"], rb=d["rb"],
                            rc=d["rc"], cr=d["cr"]))

    nc = _build(st, n_nodes)
    trace = bool(os.environ.get("GCN_TRACE"))
    try:
        res = run_bass_kernel_spmd(nc, in_maps, list(range(NCORES)), trace=trace)
    except Exception:
        if not trace:
            raise
        res = run_bass_kernel_spmd(nc, in_maps, list(range(NCORES)))
    LAST_EXEC_NS = res.exec_time_ns
    out = np.concatenate([res.results[c]["out"] for c in range(NCORES)], axis=0)
    return np.ascontiguousarray(out, dtype=np.float32)


def kernel(features, W1, b1, W2, b2, src, dst):
    return _gcn(features, W1, b1, W2, b2, src, dst)
